# revision 24
# baseline (speedup 1.0000x reference)
"""Trainium2 Bass kernel for nn_Block_62156766708387 (moe_routing).

Transformer block: x + attn(LN1(x)), then + top2-MoE(LN2(.)).

Execution plan (8 NeuronCores):
  Launch A  (data-parallel over batch, 1 batch element / core):
      attention -> x1 = x + attnout, all matmuls single-pass f16 with
      fp32 PSUM accumulation (~1e-4-class absolute error on x1).
      Scores matmuls contract over hd=64 only, so the two heads sharing
      a 128-partition block run as concurrent 64x128 row-tiles (T0/T8).
      AV runs in the yT = v^T @ p layout (free dim 512), with an
      augmented ones-column in v producing the softmax denominator as
      row 64 of the same PSUM tile; normalization = DVE reciprocal +
      a rank-1 PE broadcast + one DVE multiply per (head, qblock).
  Host:     LN2 + gate logits (fp64) from device x1.  Routing decisions
      are protected against the f16 noise by an exact-fixup: tokens
      whose 2nd-vs-3rd logit gap < THETA get their x1 row recomputed in
      fp64 on the host (exact attention row) and re-routed, so routing
      matches the reference even though the device math is 1-pass f16.
  Launch B  (expert-parallel, expert e on core e):
      fp16 FFN y = gelu(tok @ W1 + b1) @ W2 + b2 over CAP token slots.
  Host:     weighted scatter-add + residual; >CAP overflow in fp64.
"""

import numpy as np
import ml_dtypes

import concourse.bass as bass
import concourse.tile as tile
from concourse import bacc, mybir
from concourse import bass_utils
from concourse.bass import ts

F32 = mybir.dt.float32
F16 = mybir.dt.float16
BF16 = mybir.dt.bfloat16

B, T, D = 8, 1024, 1024
H = 4 * D
E = 8
NH, HD = 16, 64
EPS = 1e-5
N_CORES = 8
PT = T // 128    # 8   T tiles
PD = D // 128    # 8   D tiles
PH = H // 128    # 32  H tiles
CAP = 2048       # token slots per expert (capacity factor 1.0; the exact
                 # mean load with top-2 over 8 experts).  Overflow beyond
                 # CAP is computed exactly on the host (fp64), same
                 # mechanism as the >max-count overflow path always had.
CHUNKS = [512, 512, 512, 512]
assert sum(CHUNKS) == CAP
THETA = 4e-3     # 2nd-vs-3rd logit gap below which routing is host-fixed

_CACHE = {}


# --------------------------------------------------------------------------
# Launch A: attention block (per-core = one batch element), 1-pass f16
# --------------------------------------------------------------------------
def _build_attn(reps=1, cfg=None):
    cfg = cfg or {}
    nc = bacc.Bacc("TRN2", target_bir_lowering=False, debug=False,
                   num_devices=N_CORES)
    xp_d = nc.dram_tensor("xplus", [T, D], F32, kind="ExternalInput")
    h1_d = nc.dram_tensor("h1t", [D, T], F16, kind="ExternalInput")
    w_d = {}
    for w in ("wq", "wk", "wv", "wp"):
        w_d[w] = nc.dram_tensor(w, [D, D], F16, kind="ExternalInput")
    bq_d = nc.dram_tensor("bq8", [D], F32, kind="ExternalInput")
    bk_d = nc.dram_tensor("bk", [D], F32, kind="ExternalInput")
    msk_d = nc.dram_tensor("masks", [4, 128, 512], F16, kind="ExternalInput")
    one_d = nc.dram_tensor("onesb", [65, 65], F16, kind="ExternalInput")
    x1_d = nc.dram_tensor("x1", [T, D], F32, kind="ExternalOutput")

    xp_r = xp_d.ap().rearrange("(a p) n -> p a n", p=128)     # [128, 8, 1024]
    h1_r = h1_d.ap().rearrange("(a p) t -> p a t", p=128)
    w_r = {w: w_d[w].ap().rearrange("(k p) n -> p k n", p=128) for w in w_d}
    x1_r = x1_d.ap().rearrange("(a p) n -> p a n", p=128)

    Ident = mybir.ActivationFunctionType.Identity
    Exp = mybir.ActivationFunctionType.Exp

    with tile.TileContext(nc) as tc:
        with (
            tc.tile_pool(name="consts", bufs=1) as consts,
            tc.tile_pool(name="persist", bufs=1) as persist,
        ):
            masks = consts.tile([128, 4, 512], F16)
            nc.sync.dma_start(out=masks[:],
                              in_=msk_d.ap().rearrange("m p c -> p m c"))
            # broadcast matrix: zeros except row 64 (all ones).  Used as a
            # full-mode lhsT so out[m, q] = rcf[64, q] for every m -- a
            # partition broadcast of the reciprocal row with no PE
            # tiling-mode switch (K=65 and M=65 both round up to 128).
            onesb = consts.tile([65, 65], F16)
            nc.sync.dma_start(out=onesb[:], in_=one_d.ap())
            bq_t = consts.tile([128, PD], F32)
            nc.sync.dma_start(out=bq_t[:],
                              in_=bq_d.ap().rearrange("(a p) -> p a", p=128))
            bk_t = consts.tile([128, PD], F32)
            nc.sync.dma_start(out=bk_t[:],
                              in_=bk_d.ap().rearrange("(a p) -> p a", p=128))

            qT = persist.tile([128, PD, T], F16)
            kT = persist.tile([128, PD, T], F16)
            vaug = persist.tile([128, PT, NH, HD + 1], F16)
            zT = persist.tile([128, PD, T], F16)
            nc.gpsimd.memset(vaug[:, :, :, HD:HD + 1], 1.0)

            for rep in range(reps):
                # ---------------- QKV projections (f16 1-pass) -------------
                with (
                    tc.tile_pool(name=f"h1p{rep}", bufs=1) as h1p,
                    tc.tile_pool(name=f"wpl{rep}", bufs=1) as wpool,
                    tc.tile_pool(name=f"psC{rep}", bufs=cfg.get("psC", 4),
                                 space=bass.MemorySpace.PSUM) as psC,
                ):
                    h1 = h1p.tile([128, PD, T], F16)
                    for a in range(PD):
                        nc.sync.dma_start(out=h1[:, a, :], in_=h1_r[:, a, :])
                    wt = {}
                    for wname in ("wq", "wk", "wv"):
                        wtile = wpool.tile([128, PD, D], F16,
                                           name=f"w_{wname}_{rep}")
                        wt[wname] = wtile
                        for kk in range(PD):
                            nc.sync.dma_start(out=wtile[:, kk, :],
                                              in_=w_r[wname][:, kk, :])

                    # Q, K: feature-major [dq, t].  The two 512-column halves
                    # share the stationary w-block back-to-back so the PE
                    # reloads each weight tile once, not twice.
                    for wname, dst, b_t in (("wq", qT, bq_t), ("wk", kT, bk_t)):
                        for j in range(PD):
                            pss = [psC.tile([128, 512], F32, tag="psC",
                                            name=f"psqk{rep}_{wname}_{j}_{_n}")
                                   for _n in range(T // 512)]
                            for kk in range(PD):
                                for n in range(T // 512):
                                    nc.tensor.matmul(
                                        pss[n][:],
                                        wt[wname][:, kk, ts(j, 128)],
                                        h1[:, kk, ts(n, 512)],
                                        start=(kk == 0), stop=(kk == PD - 1))
                            for n in range(T // 512):
                                nc.scalar.activation(dst[:, j, ts(n, 512)],
                                                     pss[n][:], Ident,
                                                     bias=b_t[:, j:j + 1])
                    # V: token-major [t, dv] -> vaug (no bias; bv folded)
                    for i in range(PT):
                        pss = [psC.tile([128, 512], F32, tag="psC",
                                        name=f"psv{rep}_{i}_{_n}")
                               for _n in range(D // 512)]
                        for kk in range(PD):
                            for n in range(D // 512):
                                nc.tensor.matmul(
                                    pss[n][:], h1[:, kk, ts(i, 128)],
                                    wt["wv"][:, kk, ts(n, 512)],
                                    start=(kk == 0), stop=(kk == PD - 1))
                        for n in range(D // 512):
                            nc.scalar.copy(
                                vaug[:, i, 8 * n:8 * n + 8, 0:HD],
                                pss[n][:].rearrange("p (h c) -> p h c", h=8))

                # -------- attention: paired scores, yT-layout AV -----------
                with (
                    tc.tile_pool(name=f"es{rep}",
                                 bufs=cfg.get("es", 20)) as espool,
                    tc.tile_pool(name=f"yt{rep}",
                                 bufs=cfg.get("ytmp", 6)) as ytmp,
                    tc.tile_pool(name=f"rc{rep}", bufs=8) as rcp,
                    tc.tile_pool(name=f"psS{rep}", bufs=cfg.get("psS", 5),
                                 space=bass.MemorySpace.PSUM) as psS,
                    tc.tile_pool(name=f"psY{rep}", bufs=cfg.get("psY", 2),
                                 space=bass.MemorySpace.PSUM) as psY,
                    tc.tile_pool(name=f"psB{rep}", bufs=cfg.get("psB", 1),
                                 space=bass.MemorySpace.PSUM) as psB,
                ):
                    es_bufs = cfg.get("es", 36)
                    es_count = [0]
                    rc_bufs = 8
                    rc_count = [0]

                    def emit_scores_block(n, hj):
                        jmax = 4 * (n + 1)
                        es2 = [[], []]
                        for j in range(jmax):
                            # diagonal block r>=1: columns < 128r are fully
                            # masked; compute/exp only the live range
                            r = j - 4 * n
                            c0 = 128 * r if r >= 1 else 0
                            for half in range(2):
                                hsl = slice(64 * half, 64 * half + 64)
                                ps = psS.tile([128, 512], F32, tag="psS")
                                nc.tensor.matmul(
                                    ps[:, c0:], kT[hsl, hj, ts(j, 128)],
                                    qT[hsl, hj, 512 * n + c0:512 * (n + 1)],
                                    start=True, stop=True)
                                es = espool.tile([128, 512], F16, tag="es")
                                first_use = es_count[0] < es_bufs
                                es_count[0] += 1
                                if c0 and first_use:
                                    # stale SBUF could hold inf/nan and
                                    # mask*inf = nan
                                    nc.gpsimd.memset(es[:, 0:c0], 0.0)
                                nc.scalar.activation(es[:, c0:], ps[:, c0:],
                                                     Exp)
                                if r >= 0:
                                    nc.vector.tensor_mul(es[:], es[:],
                                                         masks[:, r, :])
                                es2[half].append(es)
                        return es2

                    def emit_av_norm(ctx):
                        n, hj, es2, jmax = (ctx["n"], ctx["hj"], ctx["es2"],
                                            ctx["jmax"])
                        psys = []
                        for half in range(2):
                            h = 2 * hj + half
                            psy = psY.tile([HD + 1, 512], F32, tag="psY")
                            for j in range(jmax):
                                nc.tensor.matmul(
                                    psy[:], vaug[:, j, h, :],
                                    es2[half][j][:],
                                    start=(j == 0), stop=(j == jmax - 1))
                            psys.append(psy)
                        for half in range(2):
                            psy = psys[half]
                            rcf = rcp.tile([65, 512], F16, tag="rc")
                            if rc_count[0] < rc_bufs:
                                # rows 0-63 must be finite (0): they meet the
                                # zero rows of onesb in the broadcast matmul
                                nc.vector.memset(rcf[0:64, :], 0.0)
                                rc_count[0] += 1
                            with nc.allow_low_precision(
                                    reason="softmax denom recip f16"):
                                nc.vector.reciprocal(rcf[64:65, :],
                                                     psy[HD:HD + 1, :])
                            psb = psB.tile([65, 512], F32, tag="psB")
                            nc.tensor.matmul(psb[:], onesb[:], rcf[:],
                                             start=True, stop=True)
                            yraw = ytmp.tile([64, 512], F16, tag="yraw")
                            nc.scalar.copy(yraw[:], psy[0:HD, :])
                            if half == 0:
                                nc.vector.tensor_mul(
                                    zT[0:64, hj, ts(n, 512)],
                                    yraw[:], psb[0:64, :])
                            else:
                                zn = ytmp.tile([64, 512], F16, tag="zn")
                                nc.vector.tensor_mul(zn[:], yraw[:],
                                                     psb[0:64, :])
                                nc.sync.dma_start(
                                    out=zT[64:128, hj, ts(n, 512)],
                                    in_=zn[:])

                    # coarse software pipeline: all scores of block b+1 are
                    # issued before AV(b) so AV never waits on exp, while
                    # the PE switches tiling mode only twice per block
                    blocks = [(n, hj) for n in range(T // 512)
                              for hj in range(NH // 2)]
                    prev = None
                    for (n, hj) in blocks:
                        es2 = emit_scores_block(n, hj)
                        ctx = {"n": n, "hj": hj, "jmax": 4 * (n + 1),
                               "es2": es2}
                        if prev is not None:
                            emit_av_norm(prev)
                        prev = ctx
                    emit_av_norm(prev)

                # ---------------- output proj + residual -------------------
                with (
                    tc.tile_pool(name=f"wp2{rep}", bufs=1) as wpool2,
                    tc.tile_pool(name=f"xr{rep}", bufs=4) as xr,
                    tc.tile_pool(name=f"xo{rep}", bufs=4) as xo,
                    tc.tile_pool(name=f"psP{rep}", bufs=cfg.get("psP", 4),
                                 space=bass.MemorySpace.PSUM) as psP,
                ):
                    wp_t = wpool2.tile([128, PD, D], F16)
                    for kk in range(PD):
                        nc.sync.dma_start(out=wp_t[:, kk, :],
                                          in_=w_r["wp"][:, kk, :])
                    for i in range(PT):
                        xts = []
                        for n in range(D // 512):
                            xt = xr.tile([128, 512], F32, tag="xt")
                            nc.sync.dma_start(out=xt[:],
                                              in_=xp_r[:, i, ts(n, 512)])
                            xts.append(xt)
                        pss = [psP.tile([128, 512], F32, tag="psP",
                                        name=f"psp{rep}_{i}_{_n}")
                               for _n in range(D // 512)]
                        for kk in range(PD):
                            for n in range(D // 512):
                                nc.tensor.matmul(
                                    pss[n][:], zT[:, kk, ts(i, 128)],
                                    wp_t[:, kk, ts(n, 512)],
                                    start=(kk == 0), stop=(kk == PD - 1))
                        for n in range(D // 512):
                            x1t = xo.tile([128, 512], F32, tag="x1t")
                            nc.vector.tensor_add(x1t[:], pss[n][:], xts[n][:])
                            nc.sync.dma_start(out=x1_r[:, i, ts(n, 512)],
                                              in_=x1t[:])

    nc.compile()
    return nc


# --------------------------------------------------------------------------
# Launch B: expert FFN (per-core = one expert), fp16
# --------------------------------------------------------------------------
def _build_expert(reps=1):
    nc = bacc.Bacc("TRN2", target_bir_lowering=False, debug=False,
                   num_devices=N_CORES)
    tokt_d = nc.dram_tensor("tokt", [D, CAP], F16, kind="ExternalInput")
    w1_d = nc.dram_tensor("w1", [D, H], F16, kind="ExternalInput")
    w2_d = nc.dram_tensor("w2", [H, D], F16, kind="ExternalInput")
    b1_d = nc.dram_tensor("b1", [H], F32, kind="ExternalInput")
    y_d = nc.dram_tensor("y", [CAP, D], F32, kind="ExternalOutput")

    tokt_r = tokt_d.ap().rearrange("(k p) c -> p k c", p=128)
    y_r = y_d.ap().rearrange("(a p) n -> p a n", p=128)

    with tile.TileContext(nc) as tc:
        with (
            tc.tile_pool(name="wpool", bufs=1) as wpool,
            tc.tile_pool(name="consts", bufs=1) as consts,
            tc.tile_pool(name="tokp", bufs=2) as tokp,
            tc.tile_pool(name="midp", bufs=1) as midp,
            tc.tile_pool(name="ysb", bufs=4) as ysbp,
            tc.tile_pool(name="psA", bufs=3, space=bass.MemorySpace.PSUM) as psA,
            tc.tile_pool(name="psB", bufs=4, space=bass.MemorySpace.PSUM) as psB,
        ):
            w1 = wpool.tile([128, PD, H], F16)
            w1r = w1_d.ap().rearrange("(k p) n -> p k n", p=128)
            for kk in range(PD):
                nc.sync.dma_start(out=w1[:, kk, :], in_=w1r[:, kk, :])
            w2 = wpool.tile([128, PH, D], F16)
            w2r = w2_d.ap().rearrange("(k p) n -> p k n", p=128)
            for kk in range(PH):
                nc.sync.dma_start(out=w2[:, kk, :], in_=w2r[:, kk, :])
            b1_t = consts.tile([128, PH], F32)
            nc.sync.dma_start(out=b1_t[:], in_=b1_d.ap().rearrange("(a p) -> p a", p=128))

            for rep in range(reps):
                for ci, cw in enumerate(CHUNKS):
                    c0 = 512 * ci
                    tokc = tokp.tile([128, PD, 512], F16, tag="tok")
                    for kk in range(PD):
                        nc.sync.dma_start(out=tokc[:, kk, :cw],
                                          in_=tokt_r[:, kk, c0:c0 + cw])
                    midc = midp.tile([128, PH, 512], F16, tag="mid")
                    for hj in range(PH):
                        ps = psA.tile([128, 512], F32)
                        for kk in range(PD):
                            nc.tensor.matmul(ps[:, :cw], w1[:, kk, ts(hj, 128)],
                                             tokc[:, kk, :cw],
                                             start=(kk == 0), stop=(kk == PD - 1))
                        nc.scalar.activation(midc[:, hj, :cw], ps[:, :cw],
                                             mybir.ActivationFunctionType.Gelu,
                                             bias=b1_t[:, hj:hj + 1])
                    # both 512-wide output halves share each stationary
                    # midc block back-to-back (one weight load, two matmuls)
                    for ti in range(cw // 128):
                        pss = [psB.tile([128, 512], F32, tag="ps2",
                                        name=f"ps2_{rep}_{ci}_{ti}_{_n}")
                               for _n in range(D // 512)]
                        for hj in range(PH):
                            for nn in range(D // 512):
                                nc.tensor.matmul(pss[nn][:],
                                                 midc[:, hj, ts(ti, 128)],
                                                 w2[:, hj, ts(nn, 512)],
                                                 start=(hj == 0),
                                                 stop=(hj == PH - 1))
                        for nn in range(D // 512):
                            ysb = ysbp.tile([128, 512], F32, tag="y")
                            nc.scalar.copy(ysb[:], pss[nn][:])
                            nc.sync.dma_start(
                                out=y_r[:, 4 * ci + ti, ts(nn, 512)],
                                in_=ysb[:])

    nc.compile()
    return nc


# --------------------------------------------------------------------------
# Host-side pieces
# --------------------------------------------------------------------------
def _layernorm64(x, g, b):
    x = x.astype(np.float64)
    mu = x.mean(axis=-1, keepdims=True)
    var = ((x - mu) ** 2).mean(axis=-1, keepdims=True)
    return ((x - mu) / np.sqrt(var + EPS)) * g + b


def _causal_masks():
    m = np.zeros((4, 128, 512), np.float16)
    p = np.arange(128)[:, None]
    c = np.arange(512)[None, :]
    for r in range(4):
        m[r] = (c - p >= r * 128).astype(np.float16)
    return m


def _gelu_exact64(x):
    from scipy.special import erf
    return 0.5 * x * (1.0 + erf(x / np.sqrt(2.0)))


def _get(name, builder):
    if name not in _CACHE:
        _CACHE[name] = builder()
    return _CACHE[name]


def _attn_host_prep(inp):
    """Shared fp64 prep: h1, folded weights/biases."""
    x = np.ascontiguousarray(inp["x"], np.float32)
    h1_64 = _layernorm64(x, inp["ln1_g"].astype(np.float64),
                         inp["ln1_b"].astype(np.float64))
    Wp64 = inp["Wp"].astype(np.float64)
    bv64 = inp["bv"].astype(np.float64)
    bp64 = inp["bp"].astype(np.float64)
    bias_out = (bv64 @ Wp64 + bp64).astype(np.float32)   # folded output bias
    return x, h1_64, bias_out


def _attn_in_maps(inp):
    x, h1_64, bias_out = _attn_host_prep(inp)
    h1 = h1_64.astype(np.float32)
    masks = _causal_masks()
    onesb = np.zeros((65, 65), np.float16)
    onesb[64, :] = 1.0
    wmap = {
        "wq": (inp["Wq"].astype(np.float32) / 8.0).astype(np.float16),
        "wk": inp["Wk"].astype(np.float16),
        "wv": inp["Wv"].astype(np.float16),
        "wp": inp["Wp"].astype(np.float16),
    }
    bq8 = (inp["bq"].astype(np.float64) / 8.0).astype(np.float32)
    bk = inp["bk"].astype(np.float32)
    in_maps = []
    for b in range(B):
        xplus = x[b] + bias_out[None, :]
        h1t = np.ascontiguousarray(h1[b].T).astype(np.float16)
        in_maps.append({
            "xplus": xplus, "h1t": h1t, **wmap,
            "bq8": bq8, "bk": bk,
            "masks": masks, "onesb": onesb,
        })
    return in_maps


def _exact_x1_rows(inp, h1_64, x, suspects_by_batch):
    """fp64 recompute of x1 rows for suspect tokens.  Returns
    {(b, p): x1_row fp64}."""
    Wq = inp["Wq"].astype(np.float64)
    Wk = inp["Wk"].astype(np.float64)
    Wv = inp["Wv"].astype(np.float64)
    Wp = inp["Wp"].astype(np.float64)
    bq = inp["bq"].astype(np.float64)
    bk = inp["bk"].astype(np.float64)
    bv = inp["bv"].astype(np.float64)
    bp = inp["bp"].astype(np.float64)
    out = {}
    for b, poss in suspects_by_batch.items():
        h1b = h1_64[b]                              # [T, D] fp64
        Kb = h1b @ Wk + bk                          # [T, D]
        Vb = h1b @ Wv + bv
        Kb = Kb.reshape(T, NH, HD)
        Vb = Vb.reshape(T, NH, HD)
        for p in poss:
            q = (h1b[p] @ Wq + bq).reshape(NH, HD)
            y = np.zeros((NH, HD))
            for hh in range(NH):
                s = Kb[:p + 1, hh, :] @ q[hh] / 8.0      # [p+1]
                s -= s.max()
                es = np.exp(s)
                y[hh] = (es @ Vb[:p + 1, hh, :]) / es.sum()
            out[(b, p)] = x[b, p].astype(np.float64) + y.reshape(D) @ Wp + bp
    return out


def kernel(**inputs):
    inp = {k: np.asarray(v) for k, v in inputs.items()}
    gate_W = inp["gate_W"].astype(np.float64)
    gate_b = inp["gate_b"].astype(np.float64)
    exp_W1 = inp["exp_W1"]
    exp_b1 = inp["exp_b1"]
    exp_W2 = inp["exp_W2"]
    exp_b2 = inp["exp_b2"]
    ln2_g = inp["ln2_g"].astype(np.float64)
    ln2_b = inp["ln2_b"].astype(np.float64)

    ncA = _get("attn", _build_attn)
    ncB = _get("expert", _build_expert)

    x, h1_64, _ = _attn_host_prep(inp)
    in_maps_a = _attn_in_maps(inp)
    res_a = bass_utils.run_bass_kernel_spmd(ncA, in_maps_a,
                                            core_ids=list(range(N_CORES)))
    x1 = np.stack([res_a.results[b]["x1"] for b in range(B)])   # [B, T, D] f32

    # ---- host routing (fp64) with exact fixup of gap-ambiguous tokens ----
    h2_64 = _layernorm64(x1, ln2_g, ln2_b)
    flat = h2_64.reshape(-1, D)                                  # [N, D] f64
    logits = flat @ gate_W + gate_b                              # [N, E] f64
    N = flat.shape[0]
    order = np.argsort(logits, axis=1)
    i1 = order[:, -1]
    i2 = order[:, -2]
    i3 = order[:, -3]
    ar = np.arange(N)
    gap23 = logits[ar, i2] - logits[ar, i3]

    suspects = np.nonzero(gap23 < THETA)[0]
    if suspects.size:
        by_batch = {}
        for t in suspects:
            by_batch.setdefault(t // T, []).append(t % T)
        exact_rows = _exact_x1_rows(inp, h1_64, x, by_batch)
        for t in suspects:
            row = exact_rows[(t // T, t % T)]
            mu = row.mean()
            var = ((row - mu) ** 2).mean()
            h2r = (row - mu) / np.sqrt(var + EPS) * ln2_g + ln2_b
            lg = h2r @ gate_W + gate_b
            o = np.argsort(lg)
            i1[t], i2[t] = o[-1], o[-2]
            logits[t] = lg

    l1 = logits[ar, i1]
    l2 = logits[ar, i2]
    e2 = np.exp(l2 - l1)
    wt1 = (1.0 / (1.0 + e2)).astype(np.float32)
    wt2 = (e2 / (1.0 + e2)).astype(np.float32)

    h2_16 = flat.astype(np.float32).astype(np.float16)
    tok_lists, wgt_lists, ovf = [], [], []
    in_maps_b = []
    for e in range(E):
        sel1 = np.nonzero(i1 == e)[0]
        sel2 = np.nonzero(i2 == e)[0]
        toks = np.concatenate([sel1, sel2])
        wgts = np.concatenate([wt1[sel1], wt2[sel2]])
        if toks.shape[0] > CAP:
            ovf.append((e, toks[CAP:], wgts[CAP:]))
            toks, wgts = toks[:CAP], wgts[:CAP]
        tok_lists.append(toks)
        wgt_lists.append(wgts)
        tokt = np.zeros((D, CAP), np.float16)
        tokt[:, :toks.shape[0]] = h2_16[toks].T
        in_maps_b.append({
            "tokt": tokt,
            "w1": exp_W1[e].astype(np.float16),
            "w2": exp_W2[e].astype(np.float16),
            "b1": exp_b1[e].astype(np.float32),
        })
    res_b = bass_utils.run_bass_kernel_spmd(ncB, in_maps_b,
                                            core_ids=list(range(N_CORES)))

    # ---- combine (b2 is folded in on the host: y from device has no b2) --
    moe = np.zeros((N, D), np.float32)
    b2f = exp_b2.astype(np.float32)
    moe += wt1[:, None] * b2f[i1] + wt2[:, None] * b2f[i2]
    for e in range(E):
        toks, wgts = tok_lists[e], wgt_lists[e]
        y = res_b.results[e]["y"][:toks.shape[0]]
        moe[toks] += wgts[:, None] * y
    for e, toks, wgts in ovf:
        t64 = flat[toks]
        mid = _gelu_exact64(t64 @ exp_W1[e].astype(np.float64)
                            + exp_b1[e].astype(np.float64))
        # no b2 here: the vectorized b2 term above covers every assignment
        yv = mid @ exp_W2[e].astype(np.float64)
        moe[toks] += wgts[:, None] * yv.astype(np.float32)

    out = x1.reshape(N, D) + moe
    return out.reshape(B, T, D).astype(np.float32)
